# revision 3
# baseline (speedup 1.0000x reference)
"""Trainium2 Bass kernel for nn_BiDecoder (bilinear GNN edge decoder).

Math:
    sr[e, b]  = ufeat[src_e] . (ifeat[dst_e] @ Ps[b])      # move transform to V side
    out[e, c] = sum_b W_combine[c, b] * sr[e, b]

Strategy ("two-GEMM sandwich", 8 NeuronCores, dst-chunk of 6250 per core):
  * Host precomputes vh[m, b, :] = ifeat[m] @ Ps[b] (node-level).  Per core,
    dst nodes are sorted by degree and grouped into strips of 64 "dict"
    entries; edges are laid out in 512-slot chunks where slot s belongs to
    dict entry (s mod 64) -- so the basis/dict selection mask is one STATIC
    [128, 512] tile shared by every chunk.
  * Host pre-gathers the u side d-major: ugT[128d, col] = ufeat[src].T --
    a contiguous fp16 stream, no on-device gathers at all.
  * Per chunk (512 edge slots):
      - TensorE GEMM1: M2[(k,b), e] = vh_b[dict_k] . u_e
        (lhsT = dct2T [128d, 128(k,b)] stationary per strip, rhs = ugT cols)
      - DVE/GpSimd: rhs2 = M2 * mask  (static one-hot, PSUM f32 in, f16 out)
      - TensorE GEMM2: out5[c, e] = sum_p Wsel[p, c] * rhs2[p, e]
        = sum_b W[c, b] * sr[e, b]  -- folds basis-select AND W_combine.
        25 chunks' out5 stack into one [125, 512] PSUM tile; one ScalarE
        copy (f32->f16) + one DMA evacuate the whole group.
  * out5 GEMMs are emitted LAG chunks behind the M2 GEMMs so the in-order
    TensorE queue never waits on the mask engine.
  * Host inverse-permutes columns back to edge order.
"""

import sys

if "/opt/trn_rl_repo" not in sys.path:
    sys.path.insert(0, "/opt/trn_rl_repo")

import numpy as np

N_CORES = 8
N_U = 100000
N_M = 50000
D = 128
NB = 2
NC_OUT = 5
CS_V = N_M // N_CORES   # 6250 dst nodes per core

ENT = 64                # dict entries per strip
Q = 8                   # slots per entry per chunk
CHUNK = ENT * Q         # 512
GROUP = 3               # chunks per out-group: 3 partition slots x 1 PSUM bank
OPART = 69              # used partitions in an out group tile (64 + 5)
NS = (CS_V + ENT - 1) // ENT  # 98 strips
LAG = 2                 # out5 emission lag (chunks)
GP_SET = (2,)           # pair idx % 4 in set -> mask on GpSimd (via ScalarE stage)
SC_SET = (0,)           # pair idx % 4 in set -> ScalarE stage + DVE mask at 2x
DCTB = 4                # strips per dct batch load
OB3 = 6                 # out groups batched per SBUF evac tile / DMA block


def _build_kernel(Cs, n_groups, cmax):
    from concourse import bacc, mybir
    from concourse.tile import TileContext

    dt = mybir.dt
    f16, f32 = dt.float16, dt.float32

    totc = int(np.sum(Cs))
    nc = bacc.Bacc(None, target_bir_lowering=False, debug=False)

    ugT_t = nc.declare_dram_parameter("ugT", [128, totc * CHUNK], f16, isOutput=False)
    nsb = ((NS + DCTB - 1) // DCTB) * DCTB
    dct_t = nc.declare_dram_parameter("dct", [128, nsb, 128], f16, isOutput=False)
    msk_t = nc.declare_dram_parameter("msk", [128, 2, CHUNK], f16, isOutput=False)
    wsel_t = nc.declare_dram_parameter("wsel", [128, NC_OUT], f16, isOutput=False)
    ng3 = (n_groups + OB3 - 1) // OB3
    out_t = nc.declare_dram_parameter(
        "out", [ng3, 3, NC_OUT, OB3, CHUNK], f16, isOutput=True
    )

    with TileContext(nc) as tc:
        with (
            tc.tile_pool(name="const", bufs=1) as cpool,
            tc.tile_pool(name="ug", bufs=5) as ugpool,
            tc.tile_pool(name="dct", bufs=3) as dpool,
            tc.tile_pool(name="rhs2", bufs=6) as rpool,
            tc.tile_pool(name="ob", bufs=2) as opool,
            tc.tile_pool(name="psM", bufs=3, space="PSUM") as pmpool,
            tc.tile_pool(name="psO", bufs=2, space="PSUM") as popool,
        ):
            msk = cpool.tile([128, 2, CHUNK], f16)
            nc.sync.dma_start(out=msk[:], in_=msk_t[:])
            wsel = cpool.tile([128, NC_OUT], f16)
            nc.sync.dma_start(out=wsel[:], in_=wsel_t[:])

            # HAM warm-up: ~20 back-to-back dummy matmuls (~4.5us sustained
            # PE busy) flip the PE clock gate to 8/8 (2.4 GHz) before the
            # pipeline starts; steady per-chunk matmuls then keep it warm.
            wpo = pmpool.tile([128, 2, CHUNK], f32, tag="m2p", name="wpo")
            for w in range(20):
                nc.tensor.matmul(
                    wpo[0:NC_OUT, w % 2, :], wsel[:], msk[:, w % 2, :],
                    start=True, stop=True,
                )

            po_box = [None]
            ob_box = [None]
            pend = []

            def emit_o5(g, rhs2):
                grp, slot = divmod(g, GROUP)
                if slot == 0:
                    po_box[0] = popool.tile([128, CHUNK], f32, tag="po", name="po")
                po = po_box[0]
                p0 = 32 * slot
                nc.tensor.matmul(
                    po[p0 : p0 + 5, :], wsel[:], rhs2,
                    start=True, stop=True,
                )
                if slot == GROUP - 1 or g == totc - 1:
                    gi = grp % OB3
                    if gi == 0:
                        ob_box[0] = opool.tile(
                            [OPART, OB3, CHUNK], f16, tag="ob", name="ob"
                        )
                    ob = ob_box[0]
                    nc.scalar.copy(out=ob[:, gi, :], in_=po[0:OPART, :])
                    if gi == OB3 - 1 or g == totc - 1:
                        g3 = grp // OB3
                        for j in range(3):
                            nc.scalar.dma_start(
                                out=out_t[g3][j],
                                in_=ob[32 * j : 32 * j + NC_OUT, :, :],
                            )

            gchunk = 0
            colofs = 0
            dct4 = None
            m2p_box = [None]

            def finish_pair(nh):
                # mask the completed pair (nh halves) in one wide op
                m2p = m2p_box[0]
                pidx = gchunk // 2
                rhs2p = rpool.tile([128, 2, CHUNK], f16, tag="rhs2p", name="rhs2p")
                sel = pidx % 4
                if sel in GP_SET:
                    # GpSimd cannot read PSUM: ScalarE stages to SBUF f16
                    m2s = rpool.tile([128, 2, CHUNK], f16, tag="m2s", name="m2s")
                    nc.scalar.copy(out=m2s[:, 0:nh, :], in_=m2p[:, 0:nh, :])
                    nc.gpsimd.tensor_mul(
                        rhs2p[:, 0:nh, :], m2s[:, 0:nh, :], msk[:, 0:nh, :]
                    )
                elif sel in SC_SET:
                    # ScalarE stages to SBUF f16 so the DVE mask runs at 2x
                    m2s = rpool.tile([128, 2, CHUNK], f16, tag="m2s", name="m2s")
                    nc.scalar.copy(out=m2s[:, 0:nh, :], in_=m2p[:, 0:nh, :])
                    nc.vector.tensor_mul(
                        rhs2p[:, 0:nh, :], m2s[:, 0:nh, :], msk[:, 0:nh, :]
                    )
                else:
                    nc.vector.tensor_mul(
                        rhs2p[:, 0:nh, :], m2p[:, 0:nh, :], msk[:, 0:nh, :]
                    )
                g0 = gchunk - nh + 1
                for h in range(nh):
                    pend.append((g0 + h, rhs2p[:, h, :]))

            for i in range(NS):
                C = int(Cs[i])
                ncols = C * CHUNK
                ug = ugpool.tile([128, cmax * CHUNK], f16, tag="ug")
                nc.sync.dma_start(
                    out=ug[:, 0:ncols], in_=ugT_t[:, colofs : colofs + ncols]
                )
                if i % DCTB == 0:
                    dct4 = dpool.tile([128, DCTB, 128], f16, tag="dct", name="dct4")
                    nc.sync.dma_start(
                        out=dct4[:], in_=dct_t[:, i : i + DCTB, :]
                    )
                dct = dct4[:, i % DCTB, :]
                for c in range(C):
                    par = gchunk % 2
                    if par == 0:
                        # drain ready O5s first: their deps are LAG pairs old,
                        # so the in-order PE queue never parks ready work
                        # behind an M2 waiting on a fresh ug DMA
                        while len(pend) > LAG + 2:
                            emit_o5(*pend.pop(0))
                        m2p_box[0] = pmpool.tile(
                            [128, 2, CHUNK], f32, tag="m2p", name="m2p"
                        )
                    nc.tensor.matmul(
                        m2p_box[0][:, par, :], dct,
                        ug[:, c * CHUNK : (c + 1) * CHUNK],
                        start=True, stop=True,
                    )
                    if par == 1:
                        finish_pair(2)
                    gchunk += 1
                colofs += ncols
            if gchunk % 2 == 1:
                gchunk -= 1
                finish_pair(1)
                gchunk += 1
            for item in pend:
                emit_o5(*item)
    nc.compile()
    return nc


def _prep(ufeat, ifeat, Ps, W_combine, src, dst):
    ufeat16 = np.vstack([ufeat.astype(np.float16), np.zeros((1, D), np.float16)])
    # vh[m, b, :] = ifeat[m] @ Ps[b]
    vh16 = np.einsum("md,bde->mbe", ifeat, Ps).astype(np.float16)  # [N_M, NB, D]

    per_core = []
    for core in range(N_CORES):
        eidx = np.nonzero(dst // CS_V == core)[0]
        ds = (dst[eidx] - core * CS_V).astype(np.int64)
        deg = np.bincount(ds, minlength=CS_V)
        order = np.argsort(-deg, kind="stable")
        rank = np.empty(CS_V, np.int64)
        rank[order] = np.arange(CS_V)
        per_core.append((eidx, ds, deg, order, rank))

    Cs = np.zeros(NS, np.int64)
    for core in range(N_CORES):
        _, _, deg, order, _ = per_core[core]
        sdeg = deg[order]
        for i in range(NS):
            w = sdeg[i * ENT : (i + 1) * ENT]
            md = int(w.max()) if len(w) else 0
            Cs[i] = max(Cs[i], (md + Q - 1) // Q)
    Cs = np.maximum(Cs, 1)
    totc = int(Cs.sum())
    tot = totc * CHUNK
    n_groups = (totc + GROUP - 1) // GROUP
    ofs_chunk = np.concatenate([[0], np.cumsum(Cs)])

    msk1 = np.zeros((128, CHUNK), np.float16)
    pp = np.arange(128)
    msk1[pp[:, None] // 2 == (np.arange(CHUNK) % ENT)[None, :]] = 1.0
    msk = np.ascontiguousarray(
        np.broadcast_to(msk1[:, None, :], (128, 2, CHUNK))
    )
    wsel = np.ascontiguousarray(
        W_combine.astype(np.float16)[:, pp % 2].T
    )  # [128, NC_OUT]

    in_maps = []
    gmaps = []
    for core in range(N_CORES):
        eidx, ds, deg, order, rank = per_core[core]
        j = rank[ds]
        perm = np.argsort(j, kind="stable")
        js = j[perm]
        grp_start = np.searchsorted(js, np.arange(CS_V), side="left")
        o = np.arange(len(js)) - grp_start[js]
        strip = js // ENT
        k = js % ENT
        col = ofs_chunk[strip] * CHUNK + (o // Q) * CHUNK + (o % Q) * ENT + k

        srcfull = np.full(tot, N_U, np.int64)  # pad -> zero row
        srcfull[col] = src[eidx[perm]]
        ugT = np.ascontiguousarray(ufeat16[srcfull].T)  # [128, tot]

        nsb = ((NS + DCTB - 1) // DCTB) * DCTB
        dct2T = np.zeros((128, nsb, 128), np.float16)
        for i in range(NS):
            nodes = order[i * ENT : (i + 1) * ENT]
            nk = len(nodes)
            blk = vh16[nodes + core * CS_V]  # [nk, NB, D]
            pidx = (np.arange(nk)[:, None] * 2 + np.arange(NB)[None, :]).reshape(-1)
            dct2T[:, i, pidx] = blk.reshape(nk * NB, D).T

        in_maps.append(
            {"ugT": ugT, "dct": dct2T, "msk": msk, "wsel": wsel}
        )
        gmaps.append((eidx[perm], col))
    return in_maps, gmaps, Cs, n_groups


def kernel(ufeat, ifeat, Ps, W_combine, src, dst, _trace=False, _res_out=None):
    from concourse.bass_utils import run_bass_kernel_spmd

    ufeat = np.asarray(ufeat, np.float32)
    ifeat = np.asarray(ifeat, np.float32)
    Ps = np.asarray(Ps, np.float32)
    W_combine = np.asarray(W_combine, np.float32)
    src = np.asarray(src).astype(np.int64)
    dst = np.asarray(dst).astype(np.int64)
    e = src.shape[0]

    in_maps, gmaps, Cs, n_groups = _prep(ufeat, ifeat, Ps, W_combine, src, dst)
    nc = _build_kernel(Cs, n_groups, int(Cs.max()))
    res = run_bass_kernel_spmd(nc, in_maps, list(range(N_CORES)), trace=_trace)
    if _res_out is not None:
        _res_out.append(res)

    out = np.empty((e, NC_OUT), np.float32)
    for core in range(N_CORES):
        epos, col = gmaps[core]
        r = res.results[core]["out"]  # [ng3, 3, NC_OUT, OB3, CHUNK] f16
        cidx = col // CHUNK
        e512 = col % CHUNK
        g = cidx // GROUP
        gslot = cidx % GROUP
        g3 = g // OB3
        gi = g % OB3
        vals = r[g3[:, None], gslot[:, None], np.arange(NC_OUT)[None, :],
                 gi[:, None], e512[:, None]]
        out[epos] = vals.astype(np.float32)
    return out
